# revision 7
# baseline (speedup 1.0000x reference)
"""DipoleInteraction message-passing kernel for 8 Trainium2 NeuronCores (v2).

Strategy (v2):
  - Pairs sharded by idx_i // 6250 (owner core of the destination atom);
    segment_sum is core-local, no collectives.
  - Within a core, pairs bucketed by 128-atom block of idx_i, sub-split by
    idx_j < 25000 (int16 gather indices), padded to uniform (L_lo, L_hi).
  - mu[idx_j] gathered via dma_gather rotated across 4 SWDGE queues.
  - Per chunk of 12 tiles (1536 pairs) all DVE ops are batched; per-pair
    scalars are stored duplicated ([...,2] innermost) so broadcasts keep the
    DVE 2x perf mode.
  - Filter net: block-diagonal W1 ([40,128]) packs two half-chunks into the
    128 ACT partitions; ssp = Exp+Ln from one ACT table (monkeypatched
    selection); b2 added via a ones-row matmul into PSUM.
  - Segment sum: one-hot matmul into PSUM per tile (batched one-hot gen).
"""
import sys

sys.path.insert(0, "/opt/trn_rl_repo")

import numpy as np

N_ATOMS = 50000
F = 64
NRBF = 20
NCORES = 8
NA = N_ATOMS // NCORES          # atoms per core
BLK = 128
NBLK = (NA + BLK - 1) // BLK    # 49 blocks; last block has 106 atoms
NAP = NBLK * BLK                # padded atoms per core (6272)
SPLIT = 25000                   # mu gather table halves (int16 index limit)
GCH = 12                        # pair tiles per chunk
QT = 384                        # pairs per quarter (W1/PSUM granularity)

_compiled = {}
LAST_RESULTS = None


def _ceil(x, m):
    return (x + m - 1) // m * m


def _patch_act_tables():
    import concourse.bacc as bacc
    if getattr(bacc, "_one_table_patch", False):
        return
    orig = bacc.get_activation_tables

    def one_table(arch):
        t = orig(arch)
        return {k: (v if k == "natural_log_exp_and_others" else set())
                for k, v in t.items()}

    bacc.get_activation_tables = one_table
    bacc._one_table_patch = True


def _build(L_lo, L_hi):
    _patch_act_tables()
    import concourse.bacc as bacc
    import concourse.mybir as mybir
    from concourse.tile import TileContext

    dt = mybir.dt
    AF = mybir.ActivationFunctionType
    OP = mybir.AluOpType

    L_blk = L_lo + L_hi
    T_blk = L_blk // 128
    T_lo = L_lo // 128
    NCH = T_blk // GCH

    nc = bacc.Bacc("TRN2", target_bir_lowering=False, debug=False,
                   num_devices=NCORES, num_swdge_queues=4)

    def register_const(dtype, value):
        t = nc.alloc_sbuf_tensor(f"const-{dtype.name}-{value}", [128, 1], dtype)
        nc.gpsimd.memset(t.ap(), value)
        nc.const_aps.aps[(dtype, value)] = t.ap()

    register_const(dt.float32, 0.5)
    nc.all_engine_barrier()

    fT = nc.dram_tensor("fT", [NBLK, 40, NCH, 2, QT], dt.bfloat16,
                        kind="ExternalInput")
    s2T = nc.dram_tensor("s2T", [NBLK, 128, T_blk], dt.float32,
                         kind="ExternalInput")
    vdT = nc.dram_tensor("vdT", [NBLK, 128, T_blk, 6], dt.bfloat16,
                         kind="ExternalInput")
    w3T = nc.dram_tensor("w3T", [NBLK, 128, T_blk, 6], dt.bfloat16,
                         kind="ExternalInput")
    aiT = nc.dram_tensor("aiT", [NBLK, 128, T_blk, 2], dt.bfloat16,
                         kind="ExternalInput")
    idxj = nc.dram_tensor("idxj", [NBLK, 128, L_blk // 16], dt.int16,
                          kind="ExternalInput")
    mu = nc.dram_tensor("mu", [N_ATOMS, 256], dt.bfloat16, kind="ExternalInput")
    muloc = nc.dram_tensor("muloc", [NAP, 192], dt.bfloat16,
                           kind="ExternalInput")
    w1d = nc.dram_tensor("w1d", [40, 128], dt.bfloat16, kind="ExternalInput")
    w2r = nc.dram_tensor("w2r", [128, F], dt.bfloat16, kind="ExternalInput")
    b2rep = nc.dram_tensor("b2rep", [1, QT], dt.bfloat16, kind="ExternalInput")
    onesr = nc.dram_tensor("onesr", [1, 128], dt.bfloat16, kind="ExternalInput")
    wtt = nc.dram_tensor("wtt", [F, F], dt.float32, kind="ExternalInput")
    b1c2 = nc.dram_tensor("b1c2", [128, 1], dt.float32, kind="ExternalInput")
    btc = nc.dram_tensor("btc", [F, 1], dt.float32, kind="ExternalInput")
    iotab = nc.dram_tensor("iotab", [128, 128], dt.bfloat16,
                           kind="ExternalInput")
    ident = nc.dram_tensor("ident", [128, 128], dt.float32,
                           kind="ExternalInput")
    out = nc.dram_tensor("out", [F, NAP], dt.float32, kind="ExternalOutput")

    gq = [0]

    with TileContext(nc) as tc:
        with tc.tile_pool(name="const", bufs=1) as cpool, \
             tc.tile_pool(name="sb", bufs=2) as pool, \
             tc.tile_pool(name="big", bufs=2) as bigpool, \
             tc.tile_pool(name="psh", bufs=2, space="PSUM") as psh, \
             tc.tile_pool(name="psw", bufs=2, space="PSUM") as psw, \
             tc.tile_pool(name="pseg", bufs=2, space="PSUM") as pseg, \
             tc.tile_pool(name="psat", bufs=1, space="PSUM") as psat:

            c_w1d = cpool.tile([40, 128], dt.bfloat16)
            nc.sync.dma_start(out=c_w1d[:], in_=w1d[:])
            c_w2r = cpool.tile([128, F], dt.bfloat16)
            nc.sync.dma_start(out=c_w2r[:], in_=w2r[:])
            c_b2 = cpool.tile([1, QT], dt.bfloat16)
            nc.sync.dma_start(out=c_b2[:], in_=b2rep[:])
            c_ones = cpool.tile([1, 128], dt.bfloat16)
            nc.sync.dma_start(out=c_ones[:], in_=onesr[:])
            c_wtt = cpool.tile([F, F], dt.float32)
            nc.sync.dma_start(out=c_wtt[:], in_=wtt[:])
            c_b1 = cpool.tile([128, 1], dt.float32)
            nc.sync.dma_start(out=c_b1[:], in_=b1c2[:])
            c_bt = cpool.tile([F, 1], dt.float32)
            nc.sync.dma_start(out=c_bt[:], in_=btc[:])
            c_iota = cpool.tile([128, 128], dt.bfloat16)
            nc.sync.dma_start(out=c_iota[:], in_=iotab[:])
            c_id = cpool.tile([128, 128], dt.float32)
            nc.sync.dma_start(out=c_id[:], in_=ident[:])

            pending = [None]

            def atom_side(b, ps_seg, mlt):
                segs = pool.tile([128, 192], dt.bfloat16, tag="segs")
                nc.scalar.copy(segs[:], ps_seg[:])
                prod = pool.tile([128, 3, F], dt.bfloat16, tag="prod")
                nc.gpsimd.tensor_tensor(
                    out=prod[:],
                    in0=segs[:].rearrange("p (d f) -> p d f", d=3),
                    in1=mlt[:].rearrange("p (d f) -> p d f", d=3),
                    op=OP.mult)
                dq1 = pool.tile([128, F], dt.bfloat16, tag="dq1")
                nc.gpsimd.tensor_tensor(
                    out=dq1[:], in0=prod[:, 0, :], in1=prod[:, 1, :],
                    op=OP.add)
                dqp = pool.tile([128, F], dt.float32, tag="dqp")
                nc.gpsimd.tensor_tensor(
                    out=dqp[:], in0=dq1[:], in1=prod[:, 2, :], op=OP.add)
                ps_t = psat.tile([F, 128], dt.float32, tag="tr")
                nc.tensor.transpose(ps_t[:], dqp[:], c_id[:])
                dqt = pool.tile([F, 128], dt.float32, tag="dqt")
                nc.scalar.copy(dqt[:], ps_t[:])
                ps_o = psat.tile([F, 128], dt.float32, tag="o")
                nc.tensor.matmul(ps_o[:], c_wtt[:], dqt[:],
                                 start=True, stop=True)
                # stable ssp: relu(z) + ln(0.5*exp(-|z|) + 0.5)
                ab = pool.tile([F, 128], dt.float32, tag="ab")
                nc.scalar.activation(ab[:], ps_o[:], AF.Abs,
                                     bias=c_bt[:], scale=1.0)
                ex2 = pool.tile([F, 128], dt.float32, tag="ex2")
                nc.scalar.activation(ex2[:], ab[:], AF.Exp, scale=-1.0)
                ln2 = pool.tile([F, 128], dt.float32, tag="ln2")
                nc.scalar.activation(ln2[:], ex2[:], AF.Ln,
                                     bias=0.5, scale=0.5)
                rl = pool.tile([F, 128], dt.float32, tag="rl")
                nc.scalar.activation(rl[:], ps_o[:], AF.Relu,
                                     bias=c_bt[:], scale=1.0)
                so = pool.tile([F, 128], dt.float32, tag="so")
                nc.gpsimd.tensor_add(so[:], rl[:], ln2[:])
                nc.sync.dma_start(out=out[:, b * 128:(b + 1) * 128],
                                  in_=so[:])

            for b in range(NBLK):
                idxt = bigpool.tile([128, L_blk // 16], dt.int16, tag="idx")
                nc.sync.dma_start(out=idxt[:], in_=idxj[b])
                mujt = bigpool.tile([128, T_blk, 256], dt.bfloat16, tag="muj")
                for (t0, n_idx, tab_ap, col0) in (
                        (0, L_lo, mu[0:SPLIT, :], 0),
                        (T_lo, L_hi, mu[SPLIT:N_ATOMS, :], L_lo // 16)):
                    off = 0
                    while off < n_idx:
                        n = min(1024, n_idx - off)
                        nc.gpsimd.dma_gather(
                            out_ap=mujt[:, t0 + off // 128:
                                        t0 + (off + n) // 128, :],
                            in_ap=tab_ap,
                            idxs_ap=idxt[:, col0 + off // 16:
                                         col0 + (off + n) // 16],
                            num_idxs=n, num_idxs_reg=n, elem_size=256,
                            queue_num=gq[0] % 4)
                        gq[0] += 1
                        off += n
                s2t = bigpool.tile([128, T_blk], dt.float32, tag="s2")
                nc.sync.dma_start(out=s2t[:], in_=s2T[b])
                vdt = bigpool.tile([128, T_blk, 3, 2], dt.bfloat16, tag="vd")
                nc.sync.dma_start(out=vdt[:],
                                  in_=vdT[b].rearrange("p t (d two) -> p t d two", d=3))
                w3t = bigpool.tile([128, T_blk, 3, 2], dt.bfloat16, tag="w3")
                nc.sync.dma_start(out=w3t[:],
                                  in_=w3T[b].rearrange("p t (d two) -> p t d two", d=3))
                ait = bigpool.tile([128, T_blk, 2], dt.bfloat16, tag="ai")
                nc.sync.dma_start(out=ait[:], in_=aiT[b])
                fTt = bigpool.tile([40, NCH, 2, QT], dt.bfloat16, tag="fT")
                nc.sync.dma_start(out=fTt[:], in_=fT[b])
                mlt = bigpool.tile([128, 192], dt.bfloat16, tag="ml")
                nc.sync.dma_start(out=mlt[:],
                                  in_=muloc[b * 128:(b + 1) * 128, :])

                ps_seg = pseg.tile([128, 192], dt.float32, tag="seg")

                for c in range(NCH):
                    hid = pool.tile([128, 2 * QT], dt.bfloat16, tag="hid")
                    for q in range(2):
                        ps_h = psh.tile([128, QT], dt.float32, tag="h")
                        nc.tensor.matmul(ps_h[:], c_w1d[:], fTt[:, c, q, :],
                                         start=True, stop=True)
                        ex = pool.tile([128, QT], dt.bfloat16, tag="ex")
                        nc.scalar.activation(ex[:], ps_h[:], AF.Exp,
                                             bias=c_b1[:], scale=1.0)
                        nc.scalar.activation(hid[:, q * QT:(q + 1) * QT],
                                             ex[:], AF.Ln, bias=0.5, scale=0.5)

                    wjs = pool.tile([128, GCH, F], dt.bfloat16, tag="wjs")
                    for h in range(2):       # PSUM halves: tiles h*6..h*6+5
                        ps_w = psw.tile([128, 6, F], dt.float32, tag="w")
                        nc.tensor.matmul(
                            ps_w[:].rearrange("p t f -> p (t f)"),
                            c_ones[:], c_b2[:],
                            start=True, stop=False, skip_group_check=True)
                        for k in range(6):
                            t = h * 6 + k
                            # hid col range for tile t of this chunk
                            part0 = 64 if t >= 6 else 0
                            tl = t - 6 if t >= 6 else t
                            col = tl * 128
                            nc.tensor.matmul(
                                ps_w[:, k, :],
                                hid[part0:part0 + 64, col:col + 128],
                                c_w2r[part0:part0 + 64, :],
                                start=False, stop=True, skip_group_check=True)
                        nc.vector.tensor_tensor(
                            out=wjs[:, h * 6:(h + 1) * 6, :], in0=ps_w[:],
                            in1=s2t[:, c * GCH + h * 6:c * GCH + (h + 1) * 6]
                                .unsqueeze(2).to_broadcast((128, 6, F)),
                            op=OP.mult)

                    muv = mujt[:, c * GCH:(c + 1) * GCH, 0:192].rearrange(
                        "p t (d f) -> p t d f", d=3)
                    vdup = vdt[:, c * GCH:(c + 1) * GCH]
                    w3dup = w3t[:, c * GCH:(c + 1) * GCH]
                    aidxd = ait[:, c * GCH:(c + 1) * GCH]

                    mw = pool.tile([128, GCH, 3, F], dt.bfloat16, tag="mw")
                    nc.vector.tensor_tensor(
                        out=mw[:], in0=muv,
                        in1=wjs[:].unsqueeze(2).to_broadcast(
                            (128, GCH, 3, F)),
                        op=OP.mult)
                    mwv = pool.tile([128, GCH, 3, F], dt.bfloat16, tag="mwv")
                    nc.vector.tensor_tensor(
                        out=mwv[:].rearrange(
                            "p t d (fh two) -> p (t d) fh two", two=2),
                        in0=mw[:].rearrange(
                            "p t d (fh two) -> p (t d) fh two", two=2),
                        in1=vdup.rearrange("p t d two -> p (t d) two")
                            .unsqueeze(2).to_broadcast(
                                (128, GCH * 3, F // 2, 2)),
                        op=OP.mult)
                    pj1 = pool.tile([128, GCH, F], dt.bfloat16, tag="pj1")
                    nc.vector.tensor_tensor(
                        out=pj1[:], in0=mwv[:, :, 0, :], in1=mwv[:, :, 1, :],
                        op=OP.add)
                    pjd = pool.tile([128, GCH, 3, F], dt.bfloat16, tag="pjd")
                    nc.vector.tensor_tensor(
                        out=pjd[:],
                        in0=pj1[:].unsqueeze(2).to_broadcast((128, GCH, 3, F)),
                        in1=mwv[:, :, 2:3, :].to_broadcast((128, GCH, 3, F)),
                        op=OP.add)
                    u = pool.tile([128, GCH, 3, F], dt.bfloat16, tag="u")
                    nc.vector.tensor_tensor(
                        out=u[:].rearrange(
                            "p t d (fh two) -> p (t d) fh two", two=2),
                        in0=pjd[:].rearrange(
                            "p t d (fh two) -> p (t d) fh two", two=2),
                        in1=w3dup.rearrange("p t d two -> p (t d) two")
                            .unsqueeze(2).to_broadcast(
                                (128, GCH * 3, F // 2, 2)),
                        op=OP.mult)
                    msg = pool.tile([128, GCH, 3, F], dt.bfloat16, tag="msg")
                    nc.vector.tensor_tensor(
                        out=msg[:], in0=mw[:], in1=u[:], op=OP.add)
                    oh = pool.tile([128, GCH, 128], dt.bfloat16, tag="oh")
                    nc.vector.tensor_tensor(
                        out=oh[:].rearrange(
                            "p t (ah two) -> p t ah two", two=2),
                        in0=c_iota[:].rearrange("p (ah two) -> p ah two",
                                                two=2)
                            .unsqueeze(1).to_broadcast((128, GCH, 64, 2)),
                        in1=aidxd.unsqueeze(2).to_broadcast(
                            (128, GCH, 64, 2)),
                        op=OP.is_equal)

                    for t in range(GCH):
                        gt = c * GCH + t
                        nc.tensor.matmul(
                            ps_seg[:], oh[:, t, :],
                            msg[:, t].rearrange("p d f -> p (d f)"),
                            start=(gt == 0), stop=(gt == T_blk - 1))

                # ---- atom side: deferred by one block ----
                if pending[0] is not None:
                    atom_side(*pending[0])
                pending[0] = (b, ps_seg, mlt)
            atom_side(*pending[0])

    nc.compile()
    return nc


def _preprocess(mu_field, f_ij, d_ij, v_ij, rcut_ij, W1, b1, W2, b2, Wt, bt,
                idx_i, idx_j):
    import ml_dtypes
    BF16 = ml_dtypes.bfloat16

    idx_i = np.asarray(idx_i).astype(np.int64).ravel()
    idx_j = np.asarray(idx_j).astype(np.int64).ravel()
    P = idx_i.shape[0]

    core = idx_i // NA
    ail = idx_i - core * NA
    blk = ail >> 7
    aidx = (ail & 127).astype(np.float32)
    jhi = (idx_j >= SPLIT).astype(np.int64)

    key = (core * NBLK + blk) * 2 + jhi
    order = np.argsort(key, kind="stable")
    cnt = np.bincount(key, minlength=NCORES * NBLK * 2)
    cnt2 = cnt.reshape(NCORES, NBLK, 2)
    L_lo = _ceil(max(int(cnt2[:, :, 0].max()), 128), 128)
    L_hi = _ceil(max(int(cnt2[:, :, 1].max()), 128), 128)
    while (L_lo + L_hi) % (GCH * 128):
        L_lo += 128
    L_blk = L_lo + L_hi
    T_blk = L_blk // 128
    NCH = T_blk // GCH
    Pc = NBLK * L_blk

    base_lo = (np.arange(NCORES * NBLK) % NBLK) * L_blk \
        + (np.arange(NCORES * NBLK) // NBLK) * Pc
    gbase = np.empty(NCORES * NBLK * 2, np.int64)
    gbase[0::2] = base_lo
    gbase[1::2] = base_lo + L_lo
    ranks = np.arange(P) - np.repeat(np.cumsum(cnt) - cnt, cnt)
    slot_g = gbase[key[order]] + ranks        # slot in [0, NCORES*Pc)
    po = order

    d = np.asarray(d_ij, np.float64).ravel()
    rc = np.asarray(rcut_ij, np.float64).ravel()
    v = np.asarray(v_ij, np.float64)
    s2 = (rc / d ** 3).astype(np.float32)
    w3 = (-3.0 * v / d[:, None] ** 2).astype(np.float32)

    # s2 (fp32) per pair
    s2A = np.zeros(NCORES * Pc, np.float32)
    s2A[slot_g] = s2[po]
    s2T = np.ascontiguousarray(
        s2A.reshape(NCORES, NBLK, T_blk, 128).transpose(0, 1, 3, 2))

    # v-dup, w3-dup, aidx-dup (bf16) per pair
    def scatterT(vals, w):
        A = np.zeros((NCORES * Pc, w), np.float32)
        A[slot_g] = vals
        return np.ascontiguousarray(
            A.reshape(NCORES, NBLK, T_blk, 128, w)
            .transpose(0, 1, 3, 2, 4)).astype(BF16)

    vdT = scatterT(np.repeat(v[po].astype(np.float32), 2, axis=1), 6)
    w3T = scatterT(np.repeat(w3[po], 2, axis=1), 6)
    aiT = scatterT(np.tile(aidx[po][:, None], (1, 2)), 2)

    # filter features, block-diag layout: [NBLK, NCH, 2, 40, QT]
    fA = np.zeros((NCORES * Pc, NRBF), np.float32)
    fA[slot_g] = np.asarray(f_ij, np.float32)[po]
    fblk = fA.reshape(NCORES, NBLK, NCH, GCH, 128, NRBF)
    # A tiles 0..5 -> [NCH, 2, 3, 128]; B tiles 6..11
    fa = fblk[:, :, :, 0:6].reshape(NCORES, NBLK, NCH, 2, QT, NRBF)
    fb = fblk[:, :, :, 6:12].reshape(NCORES, NBLK, NCH, 2, QT, NRBF)
    fT2 = np.empty((NCORES, NBLK, 40, NCH, 2, QT), np.float32)
    fT2[:, :, 0:20] = fa.transpose(0, 1, 5, 2, 3, 4)
    fT2[:, :, 20:40] = fb.transpose(0, 1, 5, 2, 3, 4)
    fT2 = np.ascontiguousarray(fT2).astype(BF16)

    jl = np.where(jhi == 1, idx_j - SPLIT, idx_j).astype(np.int16)
    iA = np.zeros(NCORES * Pc, np.int16)
    iA[slot_g] = jl[po]
    iA = iA.reshape(NCORES, NBLK, L_blk // 16, 16)
    idxJ = np.ascontiguousarray(np.tile(
        iA.transpose(0, 1, 3, 2), (1, 1, 8, 1)))   # [NC, NBLK, 128, L/16]

    mu32 = np.asarray(mu_field, np.float32).reshape(N_ATOMS, 192)
    mu_bf = np.zeros((N_ATOMS, 256), BF16)
    mu_bf[:, :192] = mu32.astype(BF16)
    muloc = np.zeros((NCORES, NAP, 192), BF16)
    muloc[:, :NA] = mu32.astype(BF16).reshape(NCORES, NA, 192)

    W1 = np.asarray(W1, np.float32)
    W2 = np.asarray(W2, np.float32)
    Wt = np.asarray(Wt, np.float32)
    b1 = np.asarray(b1, np.float32).ravel()
    b2 = np.asarray(b2, np.float32).ravel()
    bt = np.asarray(bt, np.float32).ravel()
    w1d = np.zeros((40, 128), np.float32)
    w1d[0:20, 0:64] = W1.T
    w1d[20:40, 64:128] = W1.T
    w1d = w1d.astype(BF16)
    w2r = np.tile(np.ascontiguousarray(W2.T), (2, 1)).astype(BF16)  # [128, 64]
    b2rep = np.tile(b2, 6)[None, :].astype(BF16)              # [1, 384]
    onesr = np.ones((1, 128), BF16)
    wtt = np.ascontiguousarray(Wt.T).astype(np.float32)       # [64, 64]
    b1c2 = np.concatenate([b1, b1]).reshape(128, 1).astype(np.float32)
    btcol = bt.reshape(F, 1).astype(np.float32)
    iota = np.tile(np.arange(128, dtype=np.float32), (128, 1)).astype(BF16)
    identm = np.eye(128, dtype=np.float32)

    in_maps = []
    for c in range(NCORES):
        in_maps.append({
            "fT": fT2[c], "s2T": s2T[c], "vdT": vdT[c], "w3T": w3T[c],
            "aiT": aiT[c], "idxj": idxJ[c],
            "mu": mu_bf, "muloc": muloc[c],
            "w1d": w1d, "w2r": w2r, "b2rep": b2rep, "onesr": onesr,
            "wtt": wtt, "b1c2": b1c2, "btc": btcol,
            "iotab": iota, "ident": identm,
        })
    return L_lo, L_hi, in_maps


def kernel(**inputs):
    from concourse.bass_utils import run_bass_kernel_spmd

    L_lo, L_hi, in_maps = _preprocess(
        inputs["mu_field"], inputs["f_ij"], inputs["d_ij"], inputs["v_ij"],
        inputs["rcut_ij"], inputs["W1"], inputs["b1"], inputs["W2"],
        inputs["b2"], inputs["Wt"], inputs["bt"],
        inputs["idx_i"], inputs["idx_j"])

    key = (L_lo, L_hi)
    if key not in _compiled:
        _compiled[key] = _build(L_lo, L_hi)
    nc = _compiled[key]

    res = run_bass_kernel_spmd(nc, in_maps, list(range(NCORES)))
    global LAST_RESULTS
    LAST_RESULTS = res
    dq = np.empty((N_ATOMS, 1, F), np.float32)
    for c in range(NCORES):
        o = res.results[c]["out"]            # [64, NAP]
        dq[c * NA:(c + 1) * NA, 0, :] = o[:, :NA].T
    return dq


# revision 8
# speedup vs baseline: 1.6306x; 1.6306x over previous
"""DipoleInteraction message-passing kernel for 8 Trainium2 NeuronCores (v2).

Strategy (v2):
  - Pairs sharded by idx_i // 6250 (owner core of the destination atom);
    segment_sum is core-local, no collectives.
  - Within a core, pairs bucketed by 128-atom block of idx_i, sub-split by
    idx_j < 25000 (int16 gather indices), padded to uniform (L_lo, L_hi).
  - mu[idx_j] gathered via dma_gather rotated across 4 SWDGE queues.
  - Per chunk of 12 tiles (1536 pairs) all DVE ops are batched; per-pair
    scalars are stored duplicated ([...,2] innermost) so broadcasts keep the
    DVE 2x perf mode.
  - Filter net: block-diagonal W1 ([40,128]) packs two half-chunks into the
    128 ACT partitions; ssp = Exp+Ln from one ACT table (monkeypatched
    selection); b2 added via a ones-row matmul into PSUM.
  - Segment sum: one-hot matmul into PSUM per tile (batched one-hot gen).
"""
import sys

sys.path.insert(0, "/opt/trn_rl_repo")

import numpy as np

N_ATOMS = 50000
F = 64
NRBF = 20
NCORES = 8
NA = N_ATOMS // NCORES          # atoms per core
BLK = 128
NBLK = (NA + BLK - 1) // BLK    # 49 blocks; last block has 106 atoms
NAP = NBLK * BLK                # padded atoms per core (6272)
SPLIT = 25000                   # mu gather table halves (int16 index limit)
GCH = 12                        # pair tiles per chunk
QT = 384                        # pairs per quarter (W1/PSUM granularity)

_compiled = {}
LAST_RESULTS = None


def _ceil(x, m):
    return (x + m - 1) // m * m


def _patch_act_tables():
    import concourse.bacc as bacc
    if getattr(bacc, "_one_table_patch", False):
        return
    orig = bacc.get_activation_tables

    def one_table(arch):
        t = orig(arch)
        return {k: (v if k == "natural_log_exp_and_others" else set())
                for k, v in t.items()}

    bacc.get_activation_tables = one_table
    bacc._one_table_patch = True


def _build(L_lo, L_hi):
    _patch_act_tables()
    import concourse.bacc as bacc
    import concourse.mybir as mybir
    from concourse.tile import TileContext

    dt = mybir.dt
    AF = mybir.ActivationFunctionType
    OP = mybir.AluOpType

    L_blk = L_lo + L_hi
    T_blk = L_blk // 128
    T_lo = L_lo // 128
    NCH = T_blk // GCH

    nc = bacc.Bacc("TRN2", target_bir_lowering=False, debug=False,
                   num_devices=NCORES, num_swdge_queues=4)

    def register_const(dtype, value):
        t = nc.alloc_sbuf_tensor(f"const-{dtype.name}-{value}", [128, 1], dtype)
        nc.gpsimd.memset(t.ap(), value)
        nc.const_aps.aps[(dtype, value)] = t.ap()

    register_const(dt.float32, 0.5)
    nc.all_engine_barrier()

    fT = nc.dram_tensor("fT", [NBLK, 40, NCH, 2, QT], dt.bfloat16,
                        kind="ExternalInput")
    s2T = nc.dram_tensor("s2T", [NBLK, 128, T_blk], dt.float32,
                         kind="ExternalInput")
    vdT = nc.dram_tensor("vdT", [NBLK, 128, T_blk, 6], dt.bfloat16,
                         kind="ExternalInput")
    w3T = nc.dram_tensor("w3T", [NBLK, 128, T_blk, 6], dt.bfloat16,
                         kind="ExternalInput")
    aiT = nc.dram_tensor("aiT", [NBLK, 128, T_blk, 2], dt.bfloat16,
                         kind="ExternalInput")
    idxj = nc.dram_tensor("idxj", [NBLK, 128, L_blk // 16], dt.int16,
                          kind="ExternalInput")
    mu = nc.dram_tensor("mu", [N_ATOMS, 256], dt.bfloat16, kind="ExternalInput")
    muloc = nc.dram_tensor("muloc", [NAP, 192], dt.bfloat16,
                           kind="ExternalInput")
    w1d = nc.dram_tensor("w1d", [40, 128], dt.bfloat16, kind="ExternalInput")
    w2r = nc.dram_tensor("w2r", [128, F], dt.bfloat16, kind="ExternalInput")
    b2rep = nc.dram_tensor("b2rep", [1, QT], dt.bfloat16, kind="ExternalInput")
    onesr = nc.dram_tensor("onesr", [1, 128], dt.bfloat16, kind="ExternalInput")
    wtt = nc.dram_tensor("wtt", [F, F], dt.float32, kind="ExternalInput")
    b1c2 = nc.dram_tensor("b1c2", [128, 1], dt.float32, kind="ExternalInput")
    btc = nc.dram_tensor("btc", [F, 1], dt.float32, kind="ExternalInput")
    iotab = nc.dram_tensor("iotab", [128, 128], dt.bfloat16,
                           kind="ExternalInput")
    ident = nc.dram_tensor("ident", [128, 128], dt.float32,
                           kind="ExternalInput")
    out = nc.dram_tensor("out", [F, NAP], dt.float32, kind="ExternalOutput")

    gq = [0]

    with TileContext(nc) as tc:
        with tc.tile_pool(name="const", bufs=1) as cpool, \
             tc.tile_pool(name="sb", bufs=2) as pool, \
             tc.tile_pool(name="big", bufs=2) as bigpool, \
             tc.tile_pool(name="ml", bufs=3) as mlpool, \
             tc.tile_pool(name="psh", bufs=1, space="PSUM") as psh, \
             tc.tile_pool(name="psw", bufs=4, space="PSUM") as psw, \
             tc.tile_pool(name="pseg", bufs=2, space="PSUM") as pseg, \
             tc.tile_pool(name="psat", bufs=1, space="PSUM") as psat:

            c_w1d = cpool.tile([40, 128], dt.bfloat16)
            nc.sync.dma_start(out=c_w1d[:], in_=w1d[:])
            c_w2r = cpool.tile([128, F], dt.bfloat16)
            nc.sync.dma_start(out=c_w2r[:], in_=w2r[:])
            c_b2 = cpool.tile([1, QT], dt.bfloat16)
            nc.sync.dma_start(out=c_b2[:], in_=b2rep[:])
            c_ones = cpool.tile([1, 128], dt.bfloat16)
            nc.sync.dma_start(out=c_ones[:], in_=onesr[:])
            c_wtt = cpool.tile([F, F], dt.float32)
            nc.sync.dma_start(out=c_wtt[:], in_=wtt[:])
            c_b1 = cpool.tile([128, 1], dt.float32)
            nc.sync.dma_start(out=c_b1[:], in_=b1c2[:])
            c_bt = cpool.tile([F, 1], dt.float32)
            nc.sync.dma_start(out=c_bt[:], in_=btc[:])
            c_iota = cpool.tile([128, 128], dt.bfloat16)
            nc.sync.dma_start(out=c_iota[:], in_=iotab[:])
            c_id = cpool.tile([128, 128], dt.float32)
            nc.sync.dma_start(out=c_id[:], in_=ident[:])

            pending = [None]

            def atom_side(b, ps_seg, mlt):
                prod = pool.tile([128, 3, F], dt.bfloat16, tag="prod")
                nc.vector.tensor_tensor(
                    out=prod[:],
                    in0=ps_seg[:].rearrange("p (d f) -> p d f", d=3),
                    in1=mlt[:].rearrange("p (d f) -> p d f", d=3),
                    op=OP.mult)
                dq1 = pool.tile([128, F], dt.bfloat16, tag="dq1")
                nc.vector.tensor_tensor(
                    out=dq1[:], in0=prod[:, 0, :], in1=prod[:, 1, :],
                    op=OP.add)
                dqp = pool.tile([128, F], dt.float32, tag="dqp")
                nc.vector.tensor_tensor(
                    out=dqp[:], in0=dq1[:], in1=prod[:, 2, :], op=OP.add)
                ps_at = psat.tile([F, 256], dt.float32, tag="at")
                ps_t = ps_at[:, 0:128]
                ps_o = ps_at[:, 128:256]
                nc.tensor.transpose(ps_t, dqp[:], c_id[:])
                dqt = pool.tile([F, 128], dt.float32, tag="dqt")
                nc.scalar.copy(dqt[:], ps_t)
                nc.tensor.matmul(ps_o, c_wtt[:], dqt[:],
                                 start=True, stop=True)
                # stable ssp: relu(z) + ln(0.5*exp(-|z|) + 0.5)
                ab = pool.tile([F, 128], dt.float32, tag="ab")
                nc.scalar.activation(ab[:], ps_o, AF.Abs,
                                     bias=c_bt[:], scale=1.0)
                ex2 = pool.tile([F, 128], dt.float32, tag="ex2")
                nc.scalar.activation(ex2[:], ab[:], AF.Exp, scale=-1.0)
                ln2 = pool.tile([F, 128], dt.float32, tag="ln2")
                nc.scalar.activation(ln2[:], ex2[:], AF.Ln,
                                     bias=0.5, scale=0.5)
                rl = pool.tile([F, 128], dt.float32, tag="rl")
                nc.scalar.activation(rl[:], ps_o, AF.Relu,
                                     bias=c_bt[:], scale=1.0)
                so = pool.tile([F, 128], dt.float32, tag="so")
                nc.vector.tensor_add(so[:], rl[:], ln2[:])
                nc.sync.dma_start(out=out[:, b * 128:(b + 1) * 128],
                                  in_=so[:])

            def issue_dmas(b):
                idxt = bigpool.tile([128, L_blk // 16], dt.int16, tag="idx")
                nc.sync.dma_start(out=idxt[:], in_=idxj[b])
                mujt = bigpool.tile([128, T_blk, 256], dt.bfloat16, tag="muj")
                for (t0, n_idx, tab_ap, col0) in (
                        (0, L_lo, mu[0:SPLIT, :], 0),
                        (T_lo, L_hi, mu[SPLIT:N_ATOMS, :], L_lo // 16)):
                    off = 0
                    while off < n_idx:
                        n = min(1024, n_idx - off)
                        nc.gpsimd.dma_gather(
                            out_ap=mujt[:, t0 + off // 128:
                                        t0 + (off + n) // 128, :],
                            in_ap=tab_ap,
                            idxs_ap=idxt[:, col0 + off // 16:
                                         col0 + (off + n) // 16],
                            num_idxs=n, num_idxs_reg=n, elem_size=256,
                            queue_num=gq[0] % 4)
                        gq[0] += 1
                        off += n
                s2t = bigpool.tile([128, T_blk], dt.float32, tag="s2")
                nc.sync.dma_start(out=s2t[:], in_=s2T[b])
                vdt = bigpool.tile([128, T_blk, 3, 2], dt.bfloat16, tag="vd")
                nc.sync.dma_start(out=vdt[:],
                                  in_=vdT[b].rearrange("p t (d two) -> p t d two", d=3))
                w3t = bigpool.tile([128, T_blk, 3, 2], dt.bfloat16, tag="w3")
                nc.sync.dma_start(out=w3t[:],
                                  in_=w3T[b].rearrange("p t (d two) -> p t d two", d=3))
                ait = bigpool.tile([128, T_blk, 2], dt.bfloat16, tag="ai")
                nc.sync.dma_start(out=ait[:], in_=aiT[b])
                fTt = bigpool.tile([40, NCH, 2, QT], dt.bfloat16, tag="fT")
                nc.sync.dma_start(out=fTt[:], in_=fT[b])
                mlt = mlpool.tile([128, 192], dt.bfloat16, tag="ml")
                nc.sync.dma_start(out=mlt[:],
                                  in_=muloc[b * 128:(b + 1) * 128, :])
                return dict(mujt=mujt, s2t=s2t, vdt=vdt, w3t=w3t, ait=ait,
                            fTt=fTt, mlt=mlt)

            def filter_stage(b, d):
                psws = []
                for c in range(NCH):
                    hid = pool.tile([128, 2 * QT], dt.bfloat16, tag="hid")
                    for q in range(2):
                        ps_h = psh.tile([128, QT], dt.float32, tag="h")
                        nc.tensor.matmul(ps_h[:], c_w1d[:], d["fTt"][:, c, q, :],
                                         start=True, stop=True)
                        ex = pool.tile([128, QT], dt.bfloat16, tag="ex")
                        nc.scalar.activation(ex[:], ps_h[:], AF.Exp,
                                             bias=c_b1[:], scale=1.0)
                        nc.scalar.activation(hid[:, q * QT:(q + 1) * QT],
                                             ex[:], AF.Ln, bias=0.5, scale=0.5)
                    for h in range(2):       # PSUM halves: tiles h*6..h*6+5
                        ps_w = psw.tile([128, 6, F], dt.float32, tag="w")
                        nc.tensor.matmul(
                            ps_w[:].rearrange("p t f -> p (t f)"),
                            c_ones[:], c_b2[:],
                            start=True, stop=False, skip_group_check=True)
                        for k in range(6):
                            t = h * 6 + k
                            part0 = 64 if t >= 6 else 0
                            tl = t - 6 if t >= 6 else t
                            col = tl * 128
                            nc.tensor.matmul(
                                ps_w[:, k, :],
                                hid[part0:part0 + 64, col:col + 128],
                                c_w2r[part0:part0 + 64, :],
                                start=False, stop=True, skip_group_check=True)
                        psws.append(ps_w)
                return psws

            def wjs_stage(b, d, psws):
                wjss = []
                for c in range(NCH):
                    wjs = pool.tile([128, GCH, F], dt.bfloat16, tag=f"wjs{c}")
                    for h in range(2):
                        nc.vector.tensor_tensor(
                            out=wjs[:, h * 6:(h + 1) * 6, :],
                            in0=psws[c * 2 + h][:],
                            in1=d["s2t"][:, c * GCH + h * 6:
                                         c * GCH + (h + 1) * 6]
                                .unsqueeze(2).to_broadcast((128, 6, F)),
                            op=OP.mult)
                    wjss.append(wjs)
                return wjss

            def msg_stage(b, d, wjss, ps_seg):
                mujt = d["mujt"]
                for c in range(NCH):
                    wjs = wjss[c]
                    muv = mujt[:, c * GCH:(c + 1) * GCH, 0:192].rearrange(
                        "p t (d f) -> p t d f", d=3)
                    vdup = d["vdt"][:, c * GCH:(c + 1) * GCH]
                    w3dup = d["w3t"][:, c * GCH:(c + 1) * GCH]
                    aidxd = d["ait"][:, c * GCH:(c + 1) * GCH]

                    mw = pool.tile([128, GCH, 3, F], dt.bfloat16, tag="mw")
                    nc.vector.tensor_tensor(
                        out=mw[:], in0=muv,
                        in1=wjs[:].unsqueeze(2).to_broadcast(
                            (128, GCH, 3, F)),
                        op=OP.mult)
                    mwv = pool.tile([128, GCH, 3, F], dt.bfloat16, tag="mwv")
                    nc.vector.tensor_tensor(
                        out=mwv[:].rearrange(
                            "p t d (fh two) -> p (t d) fh two", two=2),
                        in0=mw[:].rearrange(
                            "p t d (fh two) -> p (t d) fh two", two=2),
                        in1=vdup.rearrange("p t d two -> p (t d) two")
                            .unsqueeze(2).to_broadcast(
                                (128, GCH * 3, F // 2, 2)),
                        op=OP.mult)
                    pj1 = pool.tile([128, GCH, F], dt.bfloat16, tag="pj1")
                    nc.vector.tensor_tensor(
                        out=pj1[:], in0=mwv[:, :, 0, :], in1=mwv[:, :, 1, :],
                        op=OP.add)
                    pjd = pool.tile([128, GCH, 3, F], dt.bfloat16, tag="pjd")
                    nc.vector.tensor_tensor(
                        out=pjd[:],
                        in0=pj1[:].unsqueeze(2).to_broadcast((128, GCH, 3, F)),
                        in1=mwv[:, :, 2:3, :].to_broadcast((128, GCH, 3, F)),
                        op=OP.add)
                    u = pool.tile([128, GCH, 3, F], dt.bfloat16, tag="u")
                    nc.vector.tensor_tensor(
                        out=u[:].rearrange(
                            "p t d (fh two) -> p (t d) fh two", two=2),
                        in0=pjd[:].rearrange(
                            "p t d (fh two) -> p (t d) fh two", two=2),
                        in1=w3dup.rearrange("p t d two -> p (t d) two")
                            .unsqueeze(2).to_broadcast(
                                (128, GCH * 3, F // 2, 2)),
                        op=OP.mult)
                    msg = pool.tile([128, GCH, 3, F], dt.bfloat16, tag="msg")
                    nc.vector.tensor_tensor(
                        out=msg[:], in0=mw[:], in1=u[:], op=OP.add)
                    oh = pool.tile([128, GCH, 128], dt.bfloat16, tag="oh")
                    nc.vector.tensor_tensor(
                        out=oh[:].rearrange(
                            "p t (ah two) -> p t ah two", two=2),
                        in0=c_iota[:].rearrange("p (ah two) -> p ah two",
                                                two=2)
                            .unsqueeze(1).to_broadcast((128, GCH, 64, 2)),
                        in1=aidxd.unsqueeze(2).to_broadcast(
                            (128, GCH, 64, 2)),
                        op=OP.is_equal)

                    for t in range(GCH):
                        gt = c * GCH + t
                        nc.tensor.matmul(
                            ps_seg[:], oh[:, t, :],
                            msg[:, t].rearrange("p d f -> p (d f)"),
                            start=(gt == 0), stop=(gt == T_blk - 1))

            fstate = {}
            mstate = {}
            for it in range(NBLK + 2):
                if it < NBLK:
                    d = issue_dmas(it)
                    psws = filter_stage(it, d)
                    fstate[it] = (d, psws)
                if 1 <= it <= NBLK:
                    b = it - 1
                    d, psws = fstate.pop(b)
                    wjss = wjs_stage(b, d, psws)
                    ps_seg = pseg.tile([128, 192], dt.float32, tag="seg")
                    msg_stage(b, d, wjss, ps_seg)
                    mstate[b] = (ps_seg, d["mlt"])
                if it >= 2:
                    b = it - 2
                    ps_seg, mlt = mstate.pop(b)
                    atom_side(b, ps_seg, mlt)

    nc.compile()
    return nc


def _preprocess(mu_field, f_ij, d_ij, v_ij, rcut_ij, W1, b1, W2, b2, Wt, bt,
                idx_i, idx_j):
    import ml_dtypes
    BF16 = ml_dtypes.bfloat16

    idx_i = np.asarray(idx_i).astype(np.int64).ravel()
    idx_j = np.asarray(idx_j).astype(np.int64).ravel()
    P = idx_i.shape[0]

    core = idx_i // NA
    ail = idx_i - core * NA
    blk = ail >> 7
    aidx = (ail & 127).astype(np.float32)
    jhi = (idx_j >= SPLIT).astype(np.int64)

    key = (core * NBLK + blk) * 2 + jhi
    order = np.argsort(key, kind="stable")
    cnt = np.bincount(key, minlength=NCORES * NBLK * 2)
    cnt2 = cnt.reshape(NCORES, NBLK, 2)
    L_lo = _ceil(max(int(cnt2[:, :, 0].max()), 128), 128)
    L_hi = _ceil(max(int(cnt2[:, :, 1].max()), 128), 128)
    while (L_lo + L_hi) % (GCH * 128):
        L_lo += 128
    L_blk = L_lo + L_hi
    T_blk = L_blk // 128
    NCH = T_blk // GCH
    Pc = NBLK * L_blk

    base_lo = (np.arange(NCORES * NBLK) % NBLK) * L_blk \
        + (np.arange(NCORES * NBLK) // NBLK) * Pc
    gbase = np.empty(NCORES * NBLK * 2, np.int64)
    gbase[0::2] = base_lo
    gbase[1::2] = base_lo + L_lo
    ranks = np.arange(P) - np.repeat(np.cumsum(cnt) - cnt, cnt)
    slot_g = gbase[key[order]] + ranks        # slot in [0, NCORES*Pc)
    po = order

    d = np.asarray(d_ij, np.float64).ravel()
    rc = np.asarray(rcut_ij, np.float64).ravel()
    v = np.asarray(v_ij, np.float64)
    s2 = (rc / d ** 3).astype(np.float32)
    w3 = (-3.0 * v / d[:, None] ** 2).astype(np.float32)

    # s2 (fp32) per pair
    s2A = np.zeros(NCORES * Pc, np.float32)
    s2A[slot_g] = s2[po]
    s2T = np.ascontiguousarray(
        s2A.reshape(NCORES, NBLK, T_blk, 128).transpose(0, 1, 3, 2))

    # v-dup, w3-dup, aidx-dup (bf16) per pair
    def scatterT(vals, w):
        A = np.zeros((NCORES * Pc, w), np.float32)
        A[slot_g] = vals
        return np.ascontiguousarray(
            A.reshape(NCORES, NBLK, T_blk, 128, w)
            .transpose(0, 1, 3, 2, 4)).astype(BF16)

    vdT = scatterT(np.repeat(v[po].astype(np.float32), 2, axis=1), 6)
    w3T = scatterT(np.repeat(w3[po], 2, axis=1), 6)
    aiT = scatterT(np.tile(aidx[po][:, None], (1, 2)), 2)

    # filter features, block-diag layout: [NBLK, NCH, 2, 40, QT]
    fA = np.zeros((NCORES * Pc, NRBF), np.float32)
    fA[slot_g] = np.asarray(f_ij, np.float32)[po]
    fblk = fA.reshape(NCORES, NBLK, NCH, GCH, 128, NRBF)
    # A tiles 0..5 -> [NCH, 2, 3, 128]; B tiles 6..11
    fa = fblk[:, :, :, 0:6].reshape(NCORES, NBLK, NCH, 2, QT, NRBF)
    fb = fblk[:, :, :, 6:12].reshape(NCORES, NBLK, NCH, 2, QT, NRBF)
    fT2 = np.empty((NCORES, NBLK, 40, NCH, 2, QT), np.float32)
    fT2[:, :, 0:20] = fa.transpose(0, 1, 5, 2, 3, 4)
    fT2[:, :, 20:40] = fb.transpose(0, 1, 5, 2, 3, 4)
    fT2 = np.ascontiguousarray(fT2).astype(BF16)

    jl = np.where(jhi == 1, idx_j - SPLIT, idx_j).astype(np.int16)
    iA = np.zeros(NCORES * Pc, np.int16)
    iA[slot_g] = jl[po]
    iA = iA.reshape(NCORES, NBLK, L_blk // 16, 16)
    idxJ = np.ascontiguousarray(np.tile(
        iA.transpose(0, 1, 3, 2), (1, 1, 8, 1)))   # [NC, NBLK, 128, L/16]

    mu32 = np.asarray(mu_field, np.float32).reshape(N_ATOMS, 192)
    mu_bf = np.zeros((N_ATOMS, 256), BF16)
    mu_bf[:, :192] = mu32.astype(BF16)
    muloc = np.zeros((NCORES, NAP, 192), BF16)
    muloc[:, :NA] = mu32.astype(BF16).reshape(NCORES, NA, 192)

    W1 = np.asarray(W1, np.float32)
    W2 = np.asarray(W2, np.float32)
    Wt = np.asarray(Wt, np.float32)
    b1 = np.asarray(b1, np.float32).ravel()
    b2 = np.asarray(b2, np.float32).ravel()
    bt = np.asarray(bt, np.float32).ravel()
    w1d = np.zeros((40, 128), np.float32)
    w1d[0:20, 0:64] = W1.T
    w1d[20:40, 64:128] = W1.T
    w1d = w1d.astype(BF16)
    w2r = np.tile(np.ascontiguousarray(W2.T), (2, 1)).astype(BF16)  # [128, 64]
    b2rep = np.tile(b2, 6)[None, :].astype(BF16)              # [1, 384]
    onesr = np.ones((1, 128), BF16)
    wtt = np.ascontiguousarray(Wt.T).astype(np.float32)       # [64, 64]
    b1c2 = np.concatenate([b1, b1]).reshape(128, 1).astype(np.float32)
    btcol = bt.reshape(F, 1).astype(np.float32)
    iota = np.tile(np.arange(128, dtype=np.float32), (128, 1)).astype(BF16)
    identm = np.eye(128, dtype=np.float32)

    in_maps = []
    for c in range(NCORES):
        in_maps.append({
            "fT": fT2[c], "s2T": s2T[c], "vdT": vdT[c], "w3T": w3T[c],
            "aiT": aiT[c], "idxj": idxJ[c],
            "mu": mu_bf, "muloc": muloc[c],
            "w1d": w1d, "w2r": w2r, "b2rep": b2rep, "onesr": onesr,
            "wtt": wtt, "b1c2": b1c2, "btc": btcol,
            "iotab": iota, "ident": identm,
        })
    return L_lo, L_hi, in_maps


def kernel(**inputs):
    from concourse.bass_utils import run_bass_kernel_spmd

    L_lo, L_hi, in_maps = _preprocess(
        inputs["mu_field"], inputs["f_ij"], inputs["d_ij"], inputs["v_ij"],
        inputs["rcut_ij"], inputs["W1"], inputs["b1"], inputs["W2"],
        inputs["b2"], inputs["Wt"], inputs["bt"],
        inputs["idx_i"], inputs["idx_j"])

    key = (L_lo, L_hi)
    if key not in _compiled:
        _compiled[key] = _build(L_lo, L_hi)
    nc = _compiled[key]

    res = run_bass_kernel_spmd(nc, in_maps, list(range(NCORES)))
    global LAST_RESULTS
    LAST_RESULTS = res
    dq = np.empty((N_ATOMS, 1, F), np.float32)
    for c in range(NCORES):
        o = res.results[c]["out"]            # [64, NAP]
        dq[c * NA:(c + 1) * NA, 0, :] = o[:, :NA].T
    return dq


# revision 9
# speedup vs baseline: 1.7455x; 1.0704x over previous
"""DipoleInteraction message-passing kernel for 8 Trainium2 NeuronCores (v2).

Strategy (v2):
  - Pairs sharded by idx_i // 6250 (owner core of the destination atom);
    segment_sum is core-local, no collectives.
  - Within a core, pairs bucketed by 128-atom block of idx_i, sub-split by
    idx_j < 25000 (int16 gather indices), padded to uniform (L_lo, L_hi).
  - mu[idx_j] gathered via dma_gather rotated across 4 SWDGE queues.
  - Per chunk of 12 tiles (1536 pairs) all DVE ops are batched; per-pair
    scalars are stored duplicated ([...,2] innermost) so broadcasts keep the
    DVE 2x perf mode.
  - Filter net: block-diagonal W1 ([40,128]) packs two half-chunks into the
    128 ACT partitions; ssp = Exp+Ln from one ACT table (monkeypatched
    selection); b2 added via a ones-row matmul into PSUM.
  - Segment sum: one-hot matmul into PSUM per tile (batched one-hot gen).
"""
import sys

sys.path.insert(0, "/opt/trn_rl_repo")

import numpy as np

N_ATOMS = 50000
F = 64
NRBF = 20
NCORES = 8
NA = N_ATOMS // NCORES          # atoms per core
BLK = 128
NBLK = (NA + BLK - 1) // BLK    # 49 blocks; last block has 106 atoms
NAP = NBLK * BLK                # padded atoms per core (6272)
SPLIT = 25000                   # mu gather table halves (int16 index limit)
GCH = 12                        # pair tiles per chunk
QT = 384                        # pairs per quarter (W1/PSUM granularity)

_compiled = {}
LAST_RESULTS = None


def _ceil(x, m):
    return (x + m - 1) // m * m


def _patch_act_tables():
    import concourse.bacc as bacc
    if getattr(bacc, "_one_table_patch", False):
        return
    orig = bacc.get_activation_tables

    def one_table(arch):
        t = orig(arch)
        return {k: (v if k == "natural_log_exp_and_others" else set())
                for k, v in t.items()}

    bacc.get_activation_tables = one_table
    bacc._one_table_patch = True


def _build(L_lo, L_hi):
    _patch_act_tables()
    import concourse.bacc as bacc
    import concourse.mybir as mybir
    from concourse.tile import TileContext

    dt = mybir.dt
    AF = mybir.ActivationFunctionType
    OP = mybir.AluOpType

    L_blk = L_lo + L_hi
    T_blk = L_blk // 128
    T_lo = L_lo // 128
    NCH = T_blk // GCH

    nc = bacc.Bacc("TRN2", target_bir_lowering=False, debug=False,
                   num_devices=NCORES, num_swdge_queues=4)

    def register_const(dtype, value):
        t = nc.alloc_sbuf_tensor(f"const-{dtype.name}-{value}", [128, 1], dtype)
        nc.gpsimd.memset(t.ap(), value)
        nc.const_aps.aps[(dtype, value)] = t.ap()

    register_const(dt.float32, 0.5)
    nc.all_engine_barrier()

    fT = nc.dram_tensor("fT", [NBLK, 40, NCH, 2, QT], dt.bfloat16,
                        kind="ExternalInput")
    s2T = nc.dram_tensor("s2T", [NBLK, 128, T_blk], dt.float32,
                         kind="ExternalInput")
    vdT = nc.dram_tensor("vdT", [NBLK, 128, T_blk, 6], dt.bfloat16,
                         kind="ExternalInput")
    w3T = nc.dram_tensor("w3T", [NBLK, 128, T_blk, 6], dt.bfloat16,
                         kind="ExternalInput")
    aiT = nc.dram_tensor("aiT", [NBLK, 128, T_blk, 2], dt.bfloat16,
                         kind="ExternalInput")
    idxj = nc.dram_tensor("idxj", [NBLK, 128, L_blk // 16], dt.int16,
                          kind="ExternalInput")
    mu = nc.dram_tensor("mu", [N_ATOMS, 256], dt.bfloat16, kind="ExternalInput")
    muloc = nc.dram_tensor("muloc", [NAP, 192], dt.bfloat16,
                           kind="ExternalInput")
    w1d = nc.dram_tensor("w1d", [40, 128], dt.bfloat16, kind="ExternalInput")
    w2r = nc.dram_tensor("w2r", [128, F], dt.bfloat16, kind="ExternalInput")
    b2rep = nc.dram_tensor("b2rep", [1, QT], dt.bfloat16, kind="ExternalInput")
    onesr = nc.dram_tensor("onesr", [1, 128], dt.bfloat16, kind="ExternalInput")
    wtt = nc.dram_tensor("wtt", [F, F], dt.float32, kind="ExternalInput")
    b1c2 = nc.dram_tensor("b1c2", [128, 1], dt.float32, kind="ExternalInput")
    btc = nc.dram_tensor("btc", [F, 1], dt.float32, kind="ExternalInput")
    iotab = nc.dram_tensor("iotab", [128, 128], dt.bfloat16,
                           kind="ExternalInput")
    ident = nc.dram_tensor("ident", [128, 128], dt.float32,
                           kind="ExternalInput")
    out = nc.dram_tensor("out", [F, NAP], dt.float32, kind="ExternalOutput")

    gq = [0]

    with TileContext(nc) as tc:
        with tc.tile_pool(name="const", bufs=1) as cpool, \
             tc.tile_pool(name="sb", bufs=2) as pool, \
             tc.tile_pool(name="big", bufs=2) as bigpool, \
             tc.tile_pool(name="psh", bufs=2, space="PSUM") as psh, \
             tc.tile_pool(name="psw", bufs=2, space="PSUM") as psw, \
             tc.tile_pool(name="pseg", bufs=2, space="PSUM") as pseg, \
             tc.tile_pool(name="psat", bufs=1, space="PSUM") as psat:

            c_w1d = cpool.tile([40, 128], dt.bfloat16)
            nc.sync.dma_start(out=c_w1d[:], in_=w1d[:])
            c_w2r = cpool.tile([128, F], dt.bfloat16)
            nc.sync.dma_start(out=c_w2r[:], in_=w2r[:])
            c_b2 = cpool.tile([1, QT], dt.bfloat16)
            nc.sync.dma_start(out=c_b2[:], in_=b2rep[:])
            c_ones = cpool.tile([1, 128], dt.bfloat16)
            nc.sync.dma_start(out=c_ones[:], in_=onesr[:])
            c_wtt = cpool.tile([F, F], dt.float32)
            nc.sync.dma_start(out=c_wtt[:], in_=wtt[:])
            c_b1 = cpool.tile([128, 1], dt.float32)
            nc.sync.dma_start(out=c_b1[:], in_=b1c2[:])
            c_bt = cpool.tile([F, 1], dt.float32)
            nc.sync.dma_start(out=c_bt[:], in_=btc[:])
            c_iota = cpool.tile([128, 128], dt.bfloat16)
            nc.sync.dma_start(out=c_iota[:], in_=iotab[:])
            c_id = cpool.tile([128, 128], dt.float32)
            nc.sync.dma_start(out=c_id[:], in_=ident[:])

            pending = [None]

            def atom_side(b, ps_seg, mlt):
                prod = pool.tile([128, 3, F], dt.bfloat16, tag="prod")
                nc.vector.tensor_tensor(
                    out=prod[:],
                    in0=ps_seg[:].rearrange("p (d f) -> p d f", d=3),
                    in1=mlt[:].rearrange("p (d f) -> p d f", d=3),
                    op=OP.mult)
                dq1 = pool.tile([128, F], dt.bfloat16, tag="dq1")
                nc.vector.tensor_tensor(
                    out=dq1[:], in0=prod[:, 0, :], in1=prod[:, 1, :],
                    op=OP.add)
                dqp = pool.tile([128, F], dt.float32, tag="dqp")
                nc.vector.tensor_tensor(
                    out=dqp[:], in0=dq1[:], in1=prod[:, 2, :], op=OP.add)
                ps_t = psat.tile([F, 128], dt.float32, tag="tr")
                nc.tensor.transpose(ps_t[:], dqp[:], c_id[:])
                dqt = pool.tile([F, 128], dt.float32, tag="dqt")
                nc.scalar.copy(dqt[:], ps_t[:])
                ps_o = psat.tile([F, 128], dt.float32, tag="o")
                nc.tensor.matmul(ps_o[:], c_wtt[:], dqt[:],
                                 start=True, stop=True)
                # stable ssp: relu(z) + ln(0.5*exp(-|z|) + 0.5)
                ab = pool.tile([F, 128], dt.float32, tag="ab")
                nc.scalar.activation(ab[:], ps_o[:], AF.Abs,
                                     bias=c_bt[:], scale=1.0)
                ex2 = pool.tile([F, 128], dt.float32, tag="ex2")
                nc.scalar.activation(ex2[:], ab[:], AF.Exp, scale=-1.0)
                ln2 = pool.tile([F, 128], dt.float32, tag="ln2")
                nc.scalar.activation(ln2[:], ex2[:], AF.Ln,
                                     bias=0.5, scale=0.5)
                rl = pool.tile([F, 128], dt.float32, tag="rl")
                nc.scalar.activation(rl[:], ps_o[:], AF.Relu,
                                     bias=c_bt[:], scale=1.0)
                so = pool.tile([F, 128], dt.float32, tag="so")
                nc.vector.tensor_add(so[:], rl[:], ln2[:])
                nc.sync.dma_start(out=out[:, b * 128:(b + 1) * 128],
                                  in_=so[:])

            for b in range(NBLK):
                idxt = bigpool.tile([128, L_blk // 16], dt.int16, tag="idx")
                nc.sync.dma_start(out=idxt[:], in_=idxj[b])
                mujt = bigpool.tile([128, T_blk, 256], dt.bfloat16, tag="muj")
                for (t0, n_idx, tab_ap, col0) in (
                        (0, L_lo, mu[0:SPLIT, :], 0),
                        (T_lo, L_hi, mu[SPLIT:N_ATOMS, :], L_lo // 16)):
                    off = 0
                    while off < n_idx:
                        n = min(1024, n_idx - off)
                        nc.gpsimd.dma_gather(
                            out_ap=mujt[:, t0 + off // 128:
                                        t0 + (off + n) // 128, :],
                            in_ap=tab_ap,
                            idxs_ap=idxt[:, col0 + off // 16:
                                         col0 + (off + n) // 16],
                            num_idxs=n, num_idxs_reg=n, elem_size=256,
                            queue_num=gq[0] % 4)
                        gq[0] += 1
                        off += n
                s2t = bigpool.tile([128, T_blk], dt.float32, tag="s2")
                nc.sync.dma_start(out=s2t[:], in_=s2T[b])
                vdt = bigpool.tile([128, T_blk, 3, 2], dt.bfloat16, tag="vd")
                nc.sync.dma_start(out=vdt[:],
                                  in_=vdT[b].rearrange("p t (d two) -> p t d two", d=3))
                w3t = bigpool.tile([128, T_blk, 3, 2], dt.bfloat16, tag="w3")
                nc.sync.dma_start(out=w3t[:],
                                  in_=w3T[b].rearrange("p t (d two) -> p t d two", d=3))
                ait = bigpool.tile([128, T_blk, 2], dt.bfloat16, tag="ai")
                nc.sync.dma_start(out=ait[:], in_=aiT[b])
                fTt = bigpool.tile([40, NCH, 2, QT], dt.bfloat16, tag="fT")
                nc.sync.dma_start(out=fTt[:], in_=fT[b])
                mlt = bigpool.tile([128, 192], dt.bfloat16, tag="ml")
                nc.sync.dma_start(out=mlt[:],
                                  in_=muloc[b * 128:(b + 1) * 128, :])

                ps_seg = pseg.tile([128, 192], dt.float32, tag="seg")

                for c in range(NCH):
                    hid = pool.tile([128, 2 * QT], dt.bfloat16, tag="hid")
                    for q in range(2):
                        ps_h = psh.tile([128, QT], dt.float32, tag="h")
                        nc.tensor.matmul(ps_h[:], c_w1d[:], fTt[:, c, q, :],
                                         start=True, stop=True)
                        ex = pool.tile([128, QT], dt.bfloat16, tag="ex")
                        nc.scalar.activation(ex[:], ps_h[:], AF.Exp,
                                             bias=c_b1[:], scale=1.0)
                        nc.scalar.activation(hid[:, q * QT:(q + 1) * QT],
                                             ex[:], AF.Ln, bias=0.5, scale=0.5)

                    wjs = pool.tile([128, GCH, F], dt.bfloat16, tag="wjs")
                    for h in range(2):       # PSUM halves: tiles h*6..h*6+5
                        ps_w = psw.tile([128, 6, F], dt.float32, tag="w")
                        nc.tensor.matmul(
                            ps_w[:].rearrange("p t f -> p (t f)"),
                            c_ones[:], c_b2[:],
                            start=True, stop=False, skip_group_check=True)
                        for k in range(6):
                            t = h * 6 + k
                            # hid col range for tile t of this chunk
                            part0 = 64 if t >= 6 else 0
                            tl = t - 6 if t >= 6 else t
                            col = tl * 128
                            nc.tensor.matmul(
                                ps_w[:, k, :],
                                hid[part0:part0 + 64, col:col + 128],
                                c_w2r[part0:part0 + 64, :],
                                start=False, stop=True, skip_group_check=True)
                        nc.vector.tensor_tensor(
                            out=wjs[:, h * 6:(h + 1) * 6, :], in0=ps_w[:],
                            in1=s2t[:, c * GCH + h * 6:c * GCH + (h + 1) * 6]
                                .unsqueeze(2).to_broadcast((128, 6, F)),
                            op=OP.mult)

                    muv = mujt[:, c * GCH:(c + 1) * GCH, 0:192].rearrange(
                        "p t (d f) -> p t d f", d=3)
                    vdup = vdt[:, c * GCH:(c + 1) * GCH]
                    w3dup = w3t[:, c * GCH:(c + 1) * GCH]
                    aidxd = ait[:, c * GCH:(c + 1) * GCH]

                    mw = pool.tile([128, GCH, 3, F], dt.bfloat16, tag="mw")
                    nc.vector.tensor_tensor(
                        out=mw[:], in0=muv,
                        in1=wjs[:].unsqueeze(2).to_broadcast(
                            (128, GCH, 3, F)),
                        op=OP.mult)
                    mwv = pool.tile([128, GCH, 3, F], dt.bfloat16, tag="mwv")
                    nc.vector.tensor_tensor(
                        out=mwv[:].rearrange(
                            "p t d (fh two) -> p (t d) fh two", two=2),
                        in0=mw[:].rearrange(
                            "p t d (fh two) -> p (t d) fh two", two=2),
                        in1=vdup.rearrange("p t d two -> p (t d) two")
                            .unsqueeze(2).to_broadcast(
                                (128, GCH * 3, F // 2, 2)),
                        op=OP.mult)
                    pj1 = pool.tile([128, GCH, F], dt.bfloat16, tag="pj1")
                    nc.vector.tensor_tensor(
                        out=pj1[:], in0=mwv[:, :, 0, :], in1=mwv[:, :, 1, :],
                        op=OP.add)
                    pjd = pool.tile([128, GCH, 3, F], dt.bfloat16, tag="pjd")
                    nc.vector.tensor_tensor(
                        out=pjd[:],
                        in0=pj1[:].unsqueeze(2).to_broadcast((128, GCH, 3, F)),
                        in1=mwv[:, :, 2:3, :].to_broadcast((128, GCH, 3, F)),
                        op=OP.add)
                    u = pool.tile([128, GCH, 3, F], dt.bfloat16, tag="u")
                    nc.vector.tensor_tensor(
                        out=u[:].rearrange(
                            "p t d (fh two) -> p (t d) fh two", two=2),
                        in0=pjd[:].rearrange(
                            "p t d (fh two) -> p (t d) fh two", two=2),
                        in1=w3dup.rearrange("p t d two -> p (t d) two")
                            .unsqueeze(2).to_broadcast(
                                (128, GCH * 3, F // 2, 2)),
                        op=OP.mult)
                    msg = pool.tile([128, GCH, 3, F], dt.bfloat16, tag="msg")
                    nc.vector.tensor_tensor(
                        out=msg[:], in0=mw[:], in1=u[:], op=OP.add)
                    oh = pool.tile([128, GCH, 128], dt.bfloat16, tag="oh")
                    nc.vector.tensor_tensor(
                        out=oh[:].rearrange(
                            "p t (ah two) -> p t ah two", two=2),
                        in0=c_iota[:].rearrange("p (ah two) -> p ah two",
                                                two=2)
                            .unsqueeze(1).to_broadcast((128, GCH, 64, 2)),
                        in1=aidxd.unsqueeze(2).to_broadcast(
                            (128, GCH, 64, 2)),
                        op=OP.is_equal)

                    for t in range(GCH):
                        gt = c * GCH + t
                        nc.tensor.matmul(
                            ps_seg[:], oh[:, t, :],
                            msg[:, t].rearrange("p d f -> p (d f)"),
                            start=(gt == 0), stop=(gt == T_blk - 1))

                # ---- atom side: deferred by one block ----
                if pending[0] is not None:
                    atom_side(*pending[0])
                pending[0] = (b, ps_seg, mlt)
            atom_side(*pending[0])

    nc.compile()
    return nc


def _preprocess(mu_field, f_ij, d_ij, v_ij, rcut_ij, W1, b1, W2, b2, Wt, bt,
                idx_i, idx_j):
    import ml_dtypes
    BF16 = ml_dtypes.bfloat16

    idx_i = np.asarray(idx_i).astype(np.int64).ravel()
    idx_j = np.asarray(idx_j).astype(np.int64).ravel()
    P = idx_i.shape[0]

    core = idx_i // NA
    ail = idx_i - core * NA
    blk = ail >> 7
    aidx = (ail & 127).astype(np.float32)
    jhi = (idx_j >= SPLIT).astype(np.int64)

    key = (core * NBLK + blk) * 2 + jhi
    order = np.argsort(key, kind="stable")
    cnt = np.bincount(key, minlength=NCORES * NBLK * 2)
    cnt2 = cnt.reshape(NCORES, NBLK, 2)
    L_lo = _ceil(max(int(cnt2[:, :, 0].max()), 128), 128)
    L_hi = _ceil(max(int(cnt2[:, :, 1].max()), 128), 128)
    while (L_lo + L_hi) % (GCH * 128):
        L_lo += 128
    L_blk = L_lo + L_hi
    T_blk = L_blk // 128
    NCH = T_blk // GCH
    Pc = NBLK * L_blk

    base_lo = (np.arange(NCORES * NBLK) % NBLK) * L_blk \
        + (np.arange(NCORES * NBLK) // NBLK) * Pc
    gbase = np.empty(NCORES * NBLK * 2, np.int64)
    gbase[0::2] = base_lo
    gbase[1::2] = base_lo + L_lo
    ranks = np.arange(P) - np.repeat(np.cumsum(cnt) - cnt, cnt)
    slot_g = gbase[key[order]] + ranks        # slot in [0, NCORES*Pc)
    po = order

    d = np.asarray(d_ij, np.float64).ravel()
    rc = np.asarray(rcut_ij, np.float64).ravel()
    v = np.asarray(v_ij, np.float64)
    s2 = (rc / d ** 3).astype(np.float32)
    w3 = (-3.0 * v / d[:, None] ** 2).astype(np.float32)

    # s2 (fp32) per pair
    s2A = np.zeros(NCORES * Pc, np.float32)
    s2A[slot_g] = s2[po]
    s2T = np.ascontiguousarray(
        s2A.reshape(NCORES, NBLK, T_blk, 128).transpose(0, 1, 3, 2))

    # v-dup, w3-dup, aidx-dup (bf16) per pair
    def scatterT(vals, w):
        A = np.zeros((NCORES * Pc, w), np.float32)
        A[slot_g] = vals
        return np.ascontiguousarray(
            A.reshape(NCORES, NBLK, T_blk, 128, w)
            .transpose(0, 1, 3, 2, 4)).astype(BF16)

    vdT = scatterT(np.repeat(v[po].astype(np.float32), 2, axis=1), 6)
    w3T = scatterT(np.repeat(w3[po], 2, axis=1), 6)
    aiT = scatterT(np.tile(aidx[po][:, None], (1, 2)), 2)

    # filter features, block-diag layout: [NBLK, NCH, 2, 40, QT]
    fA = np.zeros((NCORES * Pc, NRBF), np.float32)
    fA[slot_g] = np.asarray(f_ij, np.float32)[po]
    fblk = fA.reshape(NCORES, NBLK, NCH, GCH, 128, NRBF)
    # A tiles 0..5 -> [NCH, 2, 3, 128]; B tiles 6..11
    fa = fblk[:, :, :, 0:6].reshape(NCORES, NBLK, NCH, 2, QT, NRBF)
    fb = fblk[:, :, :, 6:12].reshape(NCORES, NBLK, NCH, 2, QT, NRBF)
    fT2 = np.empty((NCORES, NBLK, 40, NCH, 2, QT), np.float32)
    fT2[:, :, 0:20] = fa.transpose(0, 1, 5, 2, 3, 4)
    fT2[:, :, 20:40] = fb.transpose(0, 1, 5, 2, 3, 4)
    fT2 = np.ascontiguousarray(fT2).astype(BF16)

    jl = np.where(jhi == 1, idx_j - SPLIT, idx_j).astype(np.int16)
    iA = np.zeros(NCORES * Pc, np.int16)
    iA[slot_g] = jl[po]
    iA = iA.reshape(NCORES, NBLK, L_blk // 16, 16)
    idxJ = np.ascontiguousarray(np.tile(
        iA.transpose(0, 1, 3, 2), (1, 1, 8, 1)))   # [NC, NBLK, 128, L/16]

    mu32 = np.asarray(mu_field, np.float32).reshape(N_ATOMS, 192)
    mu_bf = np.zeros((N_ATOMS, 256), BF16)
    mu_bf[:, :192] = mu32.astype(BF16)
    muloc = np.zeros((NCORES, NAP, 192), BF16)
    muloc[:, :NA] = mu32.astype(BF16).reshape(NCORES, NA, 192)

    W1 = np.asarray(W1, np.float32)
    W2 = np.asarray(W2, np.float32)
    Wt = np.asarray(Wt, np.float32)
    b1 = np.asarray(b1, np.float32).ravel()
    b2 = np.asarray(b2, np.float32).ravel()
    bt = np.asarray(bt, np.float32).ravel()
    w1d = np.zeros((40, 128), np.float32)
    w1d[0:20, 0:64] = W1.T
    w1d[20:40, 64:128] = W1.T
    w1d = w1d.astype(BF16)
    w2r = np.tile(np.ascontiguousarray(W2.T), (2, 1)).astype(BF16)  # [128, 64]
    b2rep = np.tile(b2, 6)[None, :].astype(BF16)              # [1, 384]
    onesr = np.ones((1, 128), BF16)
    wtt = np.ascontiguousarray(Wt.T).astype(np.float32)       # [64, 64]
    b1c2 = np.concatenate([b1, b1]).reshape(128, 1).astype(np.float32)
    btcol = bt.reshape(F, 1).astype(np.float32)
    iota = np.tile(np.arange(128, dtype=np.float32), (128, 1)).astype(BF16)
    identm = np.eye(128, dtype=np.float32)

    in_maps = []
    for c in range(NCORES):
        in_maps.append({
            "fT": fT2[c], "s2T": s2T[c], "vdT": vdT[c], "w3T": w3T[c],
            "aiT": aiT[c], "idxj": idxJ[c],
            "mu": mu_bf, "muloc": muloc[c],
            "w1d": w1d, "w2r": w2r, "b2rep": b2rep, "onesr": onesr,
            "wtt": wtt, "b1c2": b1c2, "btc": btcol,
            "iotab": iota, "ident": identm,
        })
    return L_lo, L_hi, in_maps


def kernel(**inputs):
    from concourse.bass_utils import run_bass_kernel_spmd

    L_lo, L_hi, in_maps = _preprocess(
        inputs["mu_field"], inputs["f_ij"], inputs["d_ij"], inputs["v_ij"],
        inputs["rcut_ij"], inputs["W1"], inputs["b1"], inputs["W2"],
        inputs["b2"], inputs["Wt"], inputs["bt"],
        inputs["idx_i"], inputs["idx_j"])

    key = (L_lo, L_hi)
    if key not in _compiled:
        _compiled[key] = _build(L_lo, L_hi)
    nc = _compiled[key]

    res = run_bass_kernel_spmd(nc, in_maps, list(range(NCORES)))
    global LAST_RESULTS
    LAST_RESULTS = res
    dq = np.empty((N_ATOMS, 1, F), np.float32)
    for c in range(NCORES):
        o = res.results[c]["out"]            # [64, NAP]
        dq[c * NA:(c + 1) * NA, 0, :] = o[:, :NA].T
    return dq
